# revision 56
# baseline (speedup 1.0000x reference)
"""Trainium2 Bass kernel for nn_Density_loss (retrieval_knn).

Computes: mean over all (row, k) of max(topk_smallest_dist(source, target)[row, k] - 0.01, 0)
where dist is the Euclidean cdist via the Gram trick (matching the reference).

Strategy (8 NeuronCores, SPMD):
  - Shard source rows across the 8 cores (1024 rows each); every core holds the
    full target set.
  - Host pre-transposes operands (fp32 has no DMA-transpose path on TRN2) and
    pre-scales source by 2 so the PE directly produces 2*s.t.
  - Per core: Gram matmul in float32r (full fp32 precision at ~1 cycle/row for
    moving dim >= 256). The -|t|^2 term is folded in two ways, balancing the
    PE and DVE under the PE clock governor: ~5/8 of groups add it on the PE
    via a K=1 rank-1 matmul into the same PSUM accumulation group; the rest
    subtract a replicated |t|^2 tile on the DVE after the PSUM->SBUF copy.
  - Top-k: nc.vector.max (top-8 per partition, descending) per half-row
    [128, 4096], then a merge max8 over the 16 half-candidates per block
    => 8 smallest d^2 per source row; keep the first top_k=5.
  - Finalize on 128x64 candidates: d2 = |s|^2 - zneg (clamped at 0), sqrt,
    hinge relu, masked sum, partition-reduce via a ones-matmul.
  - Host sums the 8 per-core partials and divides by N*top_k.
"""

import sys

import numpy as np

for _p in (
    "/root/.axon_site",
    "/root/.axon_site/_ro/trn_rl_repo",
    "/root/.axon_site/_ro/pypackages",
    "/opt/trn_rl_repo",
):
    if _p not in sys.path:
        sys.path.append(_p)

N_PTS = 8192  # source rows
M_PTS = 8192  # target rows
D = 256  # feature dim
N_CORES = 8
N_SHARD = N_PTS // N_CORES  # 1024 source rows per core
P = 128  # SBUF partitions
N_BLOCKS = N_SHARD // P  # 8 row blocks per core
CHUNK = 512  # matmul moving-operand chunk (PSUM bank)
N_CHUNKS = M_PTS // CHUNK  # 16
HALF = 8  # chunks per PSUM half-pass (8 banks)
TOP_K = 5
HINGE = 0.01

_CACHE = {}


def _build_nc(k=TOP_K):
    if k in _CACHE:
        return _CACHE[k]

    import concourse.mybir as mybir
    from concourse import bacc
    from concourse.tile import TileContext

    f32 = mybir.dt.float32
    f32r = mybir.dt.float32r
    AF = mybir.ActivationFunctionType
    ALU = mybir.AluOpType

    nc = bacc.Bacc(trn_type="TRN2", target_bir_lowering=False, debug=False)

    st = nc.dram_tensor("st", [D, N_SHARD], f32, kind="ExternalInput")
    tt = nc.dram_tensor("tt", [D, M_PTS], f32, kind="ExternalInput")
    nsqt = nc.dram_tensor("nsqt", [1, M_PTS], f32, kind="ExternalInput")
    ones1 = nc.dram_tensor("ones1", [1, P], f32, kind="ExternalInput")
    sqs = nc.dram_tensor("sqs", [P, N_BLOCKS], f32, kind="ExternalInput")
    out = nc.dram_tensor("out", [1, 1], f32, kind="ExternalOutput")

    with TileContext(nc) as tc:
        with (
            tc.tile_pool(name="persist", bufs=1) as persist,
            tc.tile_pool(name="stage", bufs=4) as stage,
            tc.tile_pool(name="small", bufs=1) as small,
            tc.tile_pool(name="psum", bufs=4, space="PSUM") as psum_pool,
        ):
            PT = 1024  # psum tile width (2 banks)
            CPT = PT // CHUNK  # chunks per psum tile
            TPH = HALF // CPT  # psum tiles per half-pass
            # Persistent operands
            tt_sb = [
                persist.tile([P, M_PTS], f32r, tag=f"tt{h}", name=f"tt{h}")
                for h in range(2)
            ]
            st_sb = [
                persist.tile([P, N_SHARD], f32r, tag=f"st{h}", name=f"st{h}")
                for h in range(2)
            ]
            nsqt_sb = persist.tile([1, M_PTS], f32r, tag="nsqt")
            ones_sb = persist.tile([1, P], f32r, tag="ones")
            onecol_sb = persist.tile([P, 1], f32, tag="onecol")
            zero_bias = persist.tile([P, 1], f32, tag="zero_bias")
            hinge_bias = persist.tile([P, 1], f32, tag="hinge_bias")
            sqs_sb = persist.tile([P, N_BLOCKS], f32, tag="sqs")
            # top-8 zneg candidates for every block
            cand = persist.tile([P, N_BLOCKS * 8], f32, tag="cand")

            # Small operands first so the first matmuls and the rank-1 fold
            # aren't queued behind 8MB of target data.
            nc.sync.dma_start(nsqt_sb[:, :], nsqt[:, :].bitcast(f32r))
            nc.sync.dma_start(ones_sb[:, :], ones1[:, :].bitcast(f32r))
            for h in range(2):
                nc.sync.dma_start(st_sb[h][:, :], st[h * P : (h + 1) * P, :].bitcast(f32r))
            nc.sync.dma_start(sqs_sb[:, :], sqs[:, :])
            nc.vector.memset(onecol_sb[:, :], 1.0)
            nc.vector.memset(zero_bias[:, :], 0.0)
            nc.vector.memset(hinge_bias[:, :], -HINGE)
            # Warm the PE (HAM clock gate) while target data streams in:
            # dummy K=1 matmuls on the small already-loaded tiles.
            warm = psum_pool.tile([P, PT], f32, tag="pz", name="warm")
            for w in range(8):
                nc.tensor.matmul(
                    warm[:, 0:CHUNK],
                    ones_sb[0:1, :],
                    nsqt_sb[0:1, 0:CHUNK],
                    start=True,
                    stop=True,
                )
            # Pre-load the ACT function tables (Sqrt/Relu) while ACT is idle.
            act_scratch = small.tile([P, 1], f32, tag="act_scratch")
            nc.scalar.activation(
                act_scratch[:, :], zero_bias[:, :], AF.Sqrt, bias=zero_bias[:, :]
            )
            nc.scalar.activation(
                act_scratch[:, :], zero_bias[:, :], AF.Relu, bias=zero_bias[:, :]
            )
            # Target halves in phase order: the whole half-0 phase (all 8
            # blocks) only needs tt[:, 0:4096], so DMA has the entire first
            # phase (~50us) to deliver the second half. No DMA-paced PE stalls
            # after the first ~1.3MB.
            # -|t|^2 replicated across all partitions for the free-dim
            # subtract (DMA broadcast: 0-stride partition read from DRAM).
            # Split in two and interleaved with the tt loads so the first
            # DVE-subtract group doesn't wait on a single 4MB transfer.
            nsqt_rep = persist.tile([P, M_PTS], f32, tag="nsqt_rep")
            for half in range(2):
                for j in range(half * 4096, half * 4096 + 4096, 2048):
                    for h in range(2):
                        nc.sync.dma_start(
                            tt_sb[h][:, j : j + 2048],
                            tt[h * P : (h + 1) * P, j : j + 2048].bitcast(f32r),
                        )
                rs = slice(half * 4096, (half + 1) * 4096)
                nc.sync.dma_start(
                    nsqt_rep[:, rs],
                    nsqt[:, rs].to_broadcast([P, 4096]),
                )

            cand16 = [
                persist.tile([P, 16], f32, tag=f"cand16_{b}", name=f"cand16_{b}")
                for b in range(N_BLOCKS)
            ]
            negd2 = small.tile([P, N_BLOCKS * 8], f32, tag="negd2")
            for half in range(N_CHUNKS // HALF):
                for b in range(N_BLOCKS):
                    bs = slice(b * P, (b + 1) * P)
                    # 5/8 of groups fold -|t|^2 on the PE (rank-1 matmul
                    # pass); the rest subtract on the DVE. Balances the two
                    # engines under the PE clock governor.
                    gi = half * N_BLOCKS + b
                    # Non-fold (DVE-subtract) groups need ~8.8us of DVE work
                    # vs ~6.9us of PE work, so spread them out -- consecutive
                    # heavy groups stall the zrow/PSUM pipeline. Half 1 gets
                    # one fewer so the DVE drains before the final block.
                    pe_fold = b not in (1, 3, 5) if half == 0 else b not in (1, 3)
                    zrow = stage.tile([P, HALF * CHUNK], f32, tag="zrow")
                    pts = [
                        psum_pool.tile([P, PT], f32, tag="pz", name=f"pz_{b}_{half}_{i}")
                        for i in range(TPH)
                    ]
                    # per-tile pass ordering: each PSUM tile finishes all its
                    # passes before the next, so ACT copies start ~4x earlier
                    for ti in range(TPH):
                        for src_h in range(2):
                            for j in range(CPT):
                                i = ti * CPT + j
                                c = half * HALF + i
                                cs = slice(c * CHUNK, (c + 1) * CHUNK)
                                nc.tensor.matmul(
                                    pts[ti][:, j * CHUNK : (j + 1) * CHUNK],
                                    st_sb[src_h][:, bs],
                                    tt_sb[src_h][:, cs],
                                    start=(src_h == 0),
                                    stop=(src_h == 1 and not pe_fold),
                                )
                        if pe_fold:
                            for j in range(CPT):
                                i = ti * CPT + j
                                c = half * HALF + i
                                cs = slice(c * CHUNK, (c + 1) * CHUNK)
                                nc.tensor.matmul(
                                    pts[ti][:, j * CHUNK : (j + 1) * CHUNK],
                                    ones_sb[0:1, :],
                                    nsqt_sb[0:1, cs],
                                    start=False,
                                    stop=True,
                                )
                    for i in range(TPH):
                        nc.scalar.copy(
                            out=zrow[:, i * PT : (i + 1) * PT], in_=pts[i][:, :]
                        )
                    if not pe_fold:
                        nc.vector.tensor_tensor(
                            out=zrow[:, :],
                            in0=zrow[:, :],
                            in1=nsqt_rep[:, half * HALF * CHUNK :][:, : HALF * CHUNK],
                            op=ALU.add,
                        )
                    # top-8 of this half-row as soon as its values land
                    if half == 1 and b >= N_BLOCKS - 4:
                        # final block: per-tile max8 so the last DVE piece is
                        # ~1.2us after the last copy instead of a 4.4us
                        # full-row max8
                        c32 = stage.tile([P, 4 * 8], f32, tag="zrow", name=f"c32_last_{b}")
                        for i in range(TPH):
                            nc.vector.max(
                                out=c32[:, i * 8 : (i + 1) * 8],
                                in_=zrow[:, i * PT : (i + 1) * PT],
                            )
                        nc.vector.max(
                            out=cand16[b][:, half * 8 : (half + 1) * 8],
                            in_=c32[:, 0 : 4 * 8],
                        )
                    else:
                        nc.vector.max(
                            out=cand16[b][:, half * 8 : (half + 1) * 8],
                            in_=zrow[:, :],
                        )
                    if half == 1:
                        # eager per-block finalize: merge halves and compute
                        # negd2 = min(zneg - |s|^2, 0) == -max(d^2, 0)
                        nc.vector.max(
                            out=cand[:, b * 8 : (b + 1) * 8],
                            in_=cand16[b][:, :],
                        )
                        nc.vector.tensor_scalar(
                            out=negd2[:, b * 8 : (b + 1) * 8],
                            in0=cand[:, b * 8 : (b + 1) * 8],
                            scalar1=sqs_sb[:, b : b + 1],
                            scalar2=0.0,
                            op0=ALU.subtract,
                            op1=ALU.min,
                        )
            # dist = sqrt(-negd2); hinge = relu(dist - HINGE)
            dist = small.tile([P, N_BLOCKS * 8], f32, tag="dist")
            nc.scalar.activation(
                dist[:, :], negd2[:, :], AF.Sqrt, bias=zero_bias[:, :], scale=-1.0
            )
            # hinge = max(d - H, 0) == max(d, H) - H, fused on the DVE to
            # avoid an extra ACT hop in the tail chain
            hinge = small.tile([P, N_BLOCKS * 8], f32, tag="hinge")
            nc.vector.tensor_scalar(
                out=hinge[:, :],
                in0=dist[:, :],
                scalar1=HINGE,
                scalar2=HINGE,
                op0=ALU.max,
                op1=ALU.subtract,
            )
            # per-partition sum of the first TOP_K of each block's 8 candidates
            hv = hinge[:, :].rearrange("p (b k) -> p b k", k=8)
            psums = small.tile([P, 1], f32, tag="psums")
            nc.vector.reduce_sum(
                psums[:, :], hv[:, :, 0:k], axis=mybir.AxisListType.XY
            )
            # partition reduce: [1,1] = psums.T @ ones
            pfin = psum_pool.tile([1, 1], f32, tag="pz")
            nc.tensor.matmul(
                pfin[:, :], psums[:, :], onecol_sb[:, :], start=True, stop=True
            )
            out_sb = small.tile([1, 1], f32, tag="outsb")
            nc.scalar.copy(out=out_sb[:, :], in_=pfin[:, :])
            nc.sync.dma_start(out[:, :], out_sb[:, :])

    nc.compile()
    _CACHE[k] = nc
    return nc


def _prep_inputs(source, target):
    src = np.asarray(source, dtype=np.float32)
    tgt = np.asarray(target, dtype=np.float32)
    st2 = np.ascontiguousarray(src.T * np.float32(2.0))  # [D, N]
    ttt = np.ascontiguousarray(tgt.T)  # [D, M]
    sqt = (tgt.astype(np.float64) ** 2).sum(axis=1).astype(np.float32)
    nsqt = np.ascontiguousarray(-sqt[None, :])  # [1, M]
    sqs = (src.astype(np.float64) ** 2).sum(axis=1).astype(np.float32)  # [N]

    in_maps = []
    for c in range(N_CORES):
        lo = c * N_SHARD
        hi = lo + N_SHARD
        sqs_c = np.ascontiguousarray(
            sqs[lo:hi].reshape(N_BLOCKS, P).T
        )  # [P, N_BLOCKS]
        in_maps.append(
            {
                "st": np.ascontiguousarray(st2[:, lo:hi]),
                "tt": ttt,
                "nsqt": nsqt,
                "ones1": np.ones((1, P), dtype=np.float32),
                "sqs": sqs_c,
            }
        )
    return in_maps


def run_spmd(in_maps, trace=False, k=TOP_K, **kwargs):
    from concourse.bass_utils import run_bass_kernel_spmd

    nc = _build_nc(k)
    return run_bass_kernel_spmd(
        nc, in_maps, list(range(N_CORES)), trace=trace, **kwargs
    )


def kernel(source, target, top_k):
    k = int(top_k)
    assert 1 <= k <= 8, f"kernel supports top_k in [1, 8], got {k}"
    in_maps = _prep_inputs(source, target)
    res = run_spmd(in_maps, k=k)
    total = float(sum(float(r["out"][0, 0]) for r in res.results))
    return np.float32(total / (N_PTS * k))


# revision 58
# speedup vs baseline: 1.1280x; 1.1280x over previous
"""Trainium2 Bass kernel for nn_Density_loss (retrieval_knn).

Computes: mean over all (row, k) of max(topk_smallest_dist(source, target)[row, k] - 0.01, 0)
where dist is the Euclidean cdist via the Gram trick (matching the reference).

Strategy (8 NeuronCores, SPMD):
  - Shard source rows across the 8 cores (1024 rows each); every core holds the
    full target set.
  - Host pre-transposes operands (fp32 has no DMA-transpose path on TRN2) and
    pre-scales source by 2 so the PE directly produces 2*s.t.
  - Per core: Gram matmul in float32r (full fp32 precision at ~1 cycle/row for
    moving dim >= 256). The -|t|^2 term is folded in two ways, balancing the
    PE and DVE under the PE clock governor: ~5/8 of groups add it on the PE
    via a K=1 rank-1 matmul into the same PSUM accumulation group; the rest
    subtract a replicated |t|^2 tile on the DVE after the PSUM->SBUF copy.
  - Top-k: nc.vector.max (top-8 per partition, descending) per half-row
    [128, 4096], then a merge max8 over the 16 half-candidates per block
    => 8 smallest d^2 per source row; keep the first top_k=5.
  - Finalize on 128x64 candidates: d2 = |s|^2 - zneg (clamped at 0), sqrt,
    hinge relu, masked sum, partition-reduce via a ones-matmul.
  - Host sums the 8 per-core partials and divides by N*top_k.
"""

import sys

import numpy as np

for _p in (
    "/root/.axon_site",
    "/root/.axon_site/_ro/trn_rl_repo",
    "/root/.axon_site/_ro/pypackages",
    "/opt/trn_rl_repo",
):
    if _p not in sys.path:
        sys.path.append(_p)

N_PTS = 8192  # source rows
M_PTS = 8192  # target rows
D = 256  # feature dim
N_CORES = 8
N_SHARD = N_PTS // N_CORES  # 1024 source rows per core
P = 128  # SBUF partitions
N_BLOCKS = N_SHARD // P  # 8 row blocks per core
CHUNK = 512  # matmul moving-operand chunk (PSUM bank)
N_CHUNKS = M_PTS // CHUNK  # 16
HALF = 8  # chunks per PSUM half-pass (8 banks)
TOP_K = 5
HINGE = 0.01

_CACHE = {}


def _build_nc(k=TOP_K):
    if k in _CACHE:
        return _CACHE[k]

    import concourse.mybir as mybir
    from concourse import bacc
    from concourse.tile import TileContext

    f32 = mybir.dt.float32
    f32r = mybir.dt.float32r
    AF = mybir.ActivationFunctionType
    ALU = mybir.AluOpType

    nc = bacc.Bacc(trn_type="TRN2", target_bir_lowering=False, debug=False)

    st = nc.dram_tensor("st", [D, N_SHARD], f32, kind="ExternalInput")
    tt = nc.dram_tensor("tt", [D, M_PTS], f32, kind="ExternalInput")
    nsqt = nc.dram_tensor("nsqt", [1, M_PTS], f32, kind="ExternalInput")
    ones1 = nc.dram_tensor("ones1", [1, P], f32, kind="ExternalInput")
    sqs = nc.dram_tensor("sqs", [P, N_BLOCKS], f32, kind="ExternalInput")
    out = nc.dram_tensor("out", [1, 1], f32, kind="ExternalOutput")

    with TileContext(nc) as tc:
        with (
            tc.tile_pool(name="persist", bufs=1) as persist,
            tc.tile_pool(name="stage", bufs=4) as stage,
            tc.tile_pool(name="small", bufs=1) as small,
            tc.tile_pool(name="psum", bufs=4, space="PSUM") as psum_pool,
        ):
            PT = 1024  # psum tile width (2 banks)
            CPT = PT // CHUNK  # chunks per psum tile
            TPH = HALF // CPT  # psum tiles per half-pass
            # Persistent operands
            tt_sb = [
                persist.tile([P, M_PTS], f32r, tag=f"tt{h}", name=f"tt{h}")
                for h in range(2)
            ]
            st_sb = [
                persist.tile([P, N_SHARD], f32r, tag=f"st{h}", name=f"st{h}")
                for h in range(2)
            ]
            nsqt_sb = persist.tile([1, M_PTS], f32r, tag="nsqt")
            ones_sb = persist.tile([1, P], f32r, tag="ones")
            onecol_sb = persist.tile([P, 1], f32, tag="onecol")
            zero_bias = persist.tile([P, 1], f32, tag="zero_bias")
            hinge_bias = persist.tile([P, 1], f32, tag="hinge_bias")
            sqs_sb = persist.tile([P, N_BLOCKS], f32, tag="sqs")
            # top-8 zneg candidates for every block
            cand = persist.tile([P, N_BLOCKS * 8], f32, tag="cand")

            # Small operands first so the first matmuls and the rank-1 fold
            # aren't queued behind 8MB of target data.
            nc.sync.dma_start(nsqt_sb[:, :], nsqt[:, :].bitcast(f32r))
            nc.sync.dma_start(ones_sb[:, :], ones1[:, :].bitcast(f32r))
            for h in range(2):
                nc.sync.dma_start(st_sb[h][:, :], st[h * P : (h + 1) * P, :].bitcast(f32r))
            nc.sync.dma_start(sqs_sb[:, :], sqs[:, :])
            nc.vector.memset(onecol_sb[:, :], 1.0)
            nc.vector.memset(zero_bias[:, :], 0.0)
            nc.vector.memset(hinge_bias[:, :], -HINGE)
            # Warm the PE (HAM clock gate) while target data streams in:
            # dummy K=1 matmuls on the small already-loaded tiles.
            warm = psum_pool.tile([P, PT], f32, tag="pz", name="warm")
            for w in range(8):
                nc.tensor.matmul(
                    warm[:, 0:CHUNK],
                    ones_sb[0:1, :],
                    nsqt_sb[0:1, 0:CHUNK],
                    start=True,
                    stop=True,
                )
            # Pre-load the ACT function tables (Sqrt/Relu) while ACT is idle.
            act_scratch = small.tile([P, 1], f32, tag="act_scratch")
            nc.scalar.activation(
                act_scratch[:, :], zero_bias[:, :], AF.Sqrt, bias=zero_bias[:, :]
            )
            nc.scalar.activation(
                act_scratch[:, :], zero_bias[:, :], AF.Relu, bias=zero_bias[:, :]
            )
            # Target halves in phase order: the whole half-0 phase (all 8
            # blocks) only needs tt[:, 0:4096], so DMA has the entire first
            # phase (~50us) to deliver the second half. No DMA-paced PE stalls
            # after the first ~1.3MB.
            # -|t|^2 replicated across all partitions for the free-dim
            # subtract (DMA broadcast: 0-stride partition read from DRAM).
            # Split in two and interleaved with the tt loads so the first
            # DVE-subtract group doesn't wait on a single 4MB transfer.
            nsqt_rep = persist.tile([P, M_PTS], f32, tag="nsqt_rep")
            for half in range(2):
                for j in range(half * 4096, half * 4096 + 4096, 2048):
                    for h in range(2):
                        nc.sync.dma_start(
                            tt_sb[h][:, j : j + 2048],
                            tt[h * P : (h + 1) * P, j : j + 2048].bitcast(f32r),
                        )
                rs = slice(half * 4096, (half + 1) * 4096)
                nc.sync.dma_start(
                    nsqt_rep[:, rs],
                    nsqt[:, rs].to_broadcast([P, 4096]),
                )

            cand16 = [
                persist.tile([P, 16], f32, tag=f"cand16_{b}", name=f"cand16_{b}")
                for b in range(N_BLOCKS)
            ]
            negd2 = small.tile([P, N_BLOCKS * 8], f32, tag="negd2")
            for half in range(N_CHUNKS // HALF):
                for b in range(N_BLOCKS):
                    bs = slice(b * P, (b + 1) * P)
                    # 5/8 of groups fold -|t|^2 on the PE (rank-1 matmul
                    # pass); the rest subtract on the DVE. Balances the two
                    # engines under the PE clock governor.
                    gi = half * N_BLOCKS + b
                    # Non-fold (DVE-subtract) groups need ~8.8us of DVE work
                    # vs ~6.9us of PE work, so spread them out -- consecutive
                    # heavy groups stall the zrow/PSUM pipeline. Half 1 gets
                    # one fewer so the DVE drains before the final block.
                    pe_fold = b not in (1, 3, 5) if half == 0 else b not in (1, 3)
                    zrow = stage.tile([P, HALF * CHUNK], f32, tag="zrow")
                    pts = [
                        psum_pool.tile([P, PT], f32, tag="pz", name=f"pz_{b}_{half}_{i}")
                        for i in range(TPH)
                    ]
                    # per-tile pass ordering: each PSUM tile finishes all its
                    # passes before the next, so ACT copies start ~4x earlier
                    for ti in range(TPH):
                        if half == 0 and b == 0 and ti > 0:
                            # group 0 is paced by the 4MB target delivery:
                            # fill the wait with dep-free warm matmuls into
                            # this tile (its real start=True resets the bank),
                            # keeping the PE array busy and the clock warm
                            for w in range(4):
                                nc.tensor.matmul(
                                    pts[ti][:, 0:CHUNK],
                                    ones_sb[0:1, :],
                                    nsqt_sb[0:1, 0:CHUNK],
                                    start=True,
                                    stop=True,
                                )
                        for src_h in range(2):
                            for j in range(CPT):
                                i = ti * CPT + j
                                c = half * HALF + i
                                cs = slice(c * CHUNK, (c + 1) * CHUNK)
                                nc.tensor.matmul(
                                    pts[ti][:, j * CHUNK : (j + 1) * CHUNK],
                                    st_sb[src_h][:, bs],
                                    tt_sb[src_h][:, cs],
                                    start=(src_h == 0),
                                    stop=(src_h == 1 and not pe_fold),
                                )
                        if pe_fold:
                            for j in range(CPT):
                                i = ti * CPT + j
                                c = half * HALF + i
                                cs = slice(c * CHUNK, (c + 1) * CHUNK)
                                nc.tensor.matmul(
                                    pts[ti][:, j * CHUNK : (j + 1) * CHUNK],
                                    ones_sb[0:1, :],
                                    nsqt_sb[0:1, cs],
                                    start=False,
                                    stop=True,
                                )
                    for i in range(TPH):
                        nc.scalar.copy(
                            out=zrow[:, i * PT : (i + 1) * PT], in_=pts[i][:, :]
                        )
                    if not pe_fold:
                        nc.vector.tensor_tensor(
                            out=zrow[:, :],
                            in0=zrow[:, :],
                            in1=nsqt_rep[:, half * HALF * CHUNK :][:, : HALF * CHUNK],
                            op=ALU.add,
                        )
                    # top-8 of this half-row as soon as its values land
                    if half == 1 and b >= N_BLOCKS - 2:
                        # final block: per-tile max8 so the last DVE piece is
                        # ~1.2us after the last copy instead of a 4.4us
                        # full-row max8
                        c32 = stage.tile([P, 4 * 8], f32, tag="zrow", name=f"c32_last_{b}")
                        for i in range(TPH):
                            nc.vector.max(
                                out=c32[:, i * 8 : (i + 1) * 8],
                                in_=zrow[:, i * PT : (i + 1) * PT],
                            )
                        nc.vector.max(
                            out=cand16[b][:, half * 8 : (half + 1) * 8],
                            in_=c32[:, 0 : 4 * 8],
                        )
                    else:
                        nc.vector.max(
                            out=cand16[b][:, half * 8 : (half + 1) * 8],
                            in_=zrow[:, :],
                        )
                    if half == 1:
                        # eager per-block finalize: merge halves and compute
                        # negd2 = min(zneg - |s|^2, 0) == -max(d^2, 0)
                        nc.vector.max(
                            out=cand[:, b * 8 : (b + 1) * 8],
                            in_=cand16[b][:, :],
                        )
                        nc.vector.tensor_scalar(
                            out=negd2[:, b * 8 : (b + 1) * 8],
                            in0=cand[:, b * 8 : (b + 1) * 8],
                            scalar1=sqs_sb[:, b : b + 1],
                            scalar2=0.0,
                            op0=ALU.subtract,
                            op1=ALU.min,
                        )
            # dist = sqrt(-negd2); hinge = relu(dist - HINGE)
            dist = small.tile([P, N_BLOCKS * 8], f32, tag="dist")
            nc.scalar.activation(
                dist[:, :], negd2[:, :], AF.Sqrt, bias=zero_bias[:, :], scale=-1.0
            )
            # hinge = max(d - H, 0) == max(d, H) - H, fused on the DVE to
            # avoid an extra ACT hop in the tail chain
            hinge = small.tile([P, N_BLOCKS * 8], f32, tag="hinge")
            nc.vector.tensor_scalar(
                out=hinge[:, :],
                in0=dist[:, :],
                scalar1=HINGE,
                scalar2=HINGE,
                op0=ALU.max,
                op1=ALU.subtract,
            )
            # per-partition sum of the first TOP_K of each block's 8 candidates
            hv = hinge[:, :].rearrange("p (b k) -> p b k", k=8)
            psums = small.tile([P, 1], f32, tag="psums")
            nc.vector.reduce_sum(
                psums[:, :], hv[:, :, 0:k], axis=mybir.AxisListType.XY
            )
            # partition reduce: [1,1] = psums.T @ ones
            pfin = psum_pool.tile([1, 1], f32, tag="pz")
            nc.tensor.matmul(
                pfin[:, :], psums[:, :], onecol_sb[:, :], start=True, stop=True
            )
            out_sb = small.tile([1, 1], f32, tag="outsb")
            nc.scalar.copy(out=out_sb[:, :], in_=pfin[:, :])
            nc.sync.dma_start(out[:, :], out_sb[:, :])

    nc.compile()
    _CACHE[k] = nc
    return nc


def _prep_inputs(source, target):
    src = np.asarray(source, dtype=np.float32)
    tgt = np.asarray(target, dtype=np.float32)
    st2 = np.ascontiguousarray(src.T * np.float32(2.0))  # [D, N]
    ttt = np.ascontiguousarray(tgt.T)  # [D, M]
    sqt = (tgt.astype(np.float64) ** 2).sum(axis=1).astype(np.float32)
    nsqt = np.ascontiguousarray(-sqt[None, :])  # [1, M]
    sqs = (src.astype(np.float64) ** 2).sum(axis=1).astype(np.float32)  # [N]

    in_maps = []
    for c in range(N_CORES):
        lo = c * N_SHARD
        hi = lo + N_SHARD
        sqs_c = np.ascontiguousarray(
            sqs[lo:hi].reshape(N_BLOCKS, P).T
        )  # [P, N_BLOCKS]
        in_maps.append(
            {
                "st": np.ascontiguousarray(st2[:, lo:hi]),
                "tt": ttt,
                "nsqt": nsqt,
                "ones1": np.ones((1, P), dtype=np.float32),
                "sqs": sqs_c,
            }
        )
    return in_maps


def run_spmd(in_maps, trace=False, k=TOP_K, **kwargs):
    from concourse.bass_utils import run_bass_kernel_spmd

    nc = _build_nc(k)
    return run_bass_kernel_spmd(
        nc, in_maps, list(range(N_CORES)), trace=trace, **kwargs
    )


def kernel(source, target, top_k):
    k = int(top_k)
    assert 1 <= k <= 8, f"kernel supports top_k in [1, 8], got {k}"
    in_maps = _prep_inputs(source, target)
    res = run_spmd(in_maps, k=k)
    total = float(sum(float(r["out"][0, 0]) for r in res.results))
    return np.float32(total / (N_PTS * k))


# revision 61
# speedup vs baseline: 1.1891x; 1.0541x over previous
"""Trainium2 Bass kernel for nn_Density_loss (retrieval_knn).

Computes: mean over all (row, k) of max(topk_smallest_dist(source, target)[row, k] - 0.01, 0)
where dist is the Euclidean cdist via the Gram trick (matching the reference).

Strategy (8 NeuronCores, SPMD):
  - Shard source rows across the 8 cores (1024 rows each); every core holds the
    full target set.
  - Host pre-transposes operands (fp32 has no DMA-transpose path on TRN2) and
    pre-scales source by 2 so the PE directly produces 2*s.t.
  - Per core: Gram matmul in float32r (full fp32 precision at ~1 cycle/row for
    moving dim >= 256). The -|t|^2 term is folded in two ways, balancing the
    PE and DVE under the PE clock governor: ~5/8 of groups add it on the PE
    via a K=1 rank-1 matmul into the same PSUM accumulation group; the rest
    subtract a replicated |t|^2 tile on the DVE after the PSUM->SBUF copy.
  - Top-k: nc.vector.max (top-8 per partition, descending) per half-row
    [128, 4096], then a merge max8 over the 16 half-candidates per block
    => 8 smallest d^2 per source row; keep the first top_k=5.
  - Finalize on 128x64 candidates: d2 = |s|^2 - zneg (clamped at 0), sqrt,
    hinge relu, masked sum, partition-reduce via a ones-matmul.
  - Host sums the 8 per-core partials and divides by N*top_k.
"""

import sys

import numpy as np

for _p in (
    "/root/.axon_site",
    "/root/.axon_site/_ro/trn_rl_repo",
    "/root/.axon_site/_ro/pypackages",
    "/opt/trn_rl_repo",
):
    if _p not in sys.path:
        sys.path.append(_p)

N_PTS = 8192  # source rows
M_PTS = 8192  # target rows
D = 256  # feature dim
N_CORES = 8
N_SHARD = N_PTS // N_CORES  # 1024 source rows per core
P = 128  # SBUF partitions
N_BLOCKS = N_SHARD // P  # 8 row blocks per core
CHUNK = 512  # matmul moving-operand chunk (PSUM bank)
N_CHUNKS = M_PTS // CHUNK  # 16
HALF = 8  # chunks per PSUM half-pass (8 banks)
TOP_K = 5
HINGE = 0.01

_CACHE = {}


def _build_nc(k=TOP_K):
    if k in _CACHE:
        return _CACHE[k]

    import concourse.mybir as mybir
    from concourse import bacc
    from concourse.tile import TileContext

    f32 = mybir.dt.float32
    f32r = mybir.dt.float32r
    AF = mybir.ActivationFunctionType
    ALU = mybir.AluOpType

    nc = bacc.Bacc(trn_type="TRN2", target_bir_lowering=False, debug=False)

    st = nc.dram_tensor("st", [D, N_SHARD], f32, kind="ExternalInput")
    tt = nc.dram_tensor("tt", [D, M_PTS], f32, kind="ExternalInput")
    nsqt = nc.dram_tensor("nsqt", [1, M_PTS], f32, kind="ExternalInput")
    ones1 = nc.dram_tensor("ones1", [1, P], f32, kind="ExternalInput")
    sqs = nc.dram_tensor("sqs", [P, N_BLOCKS], f32, kind="ExternalInput")
    out = nc.dram_tensor("out", [1, 1], f32, kind="ExternalOutput")

    with TileContext(nc) as tc:
        with (
            tc.tile_pool(name="persist", bufs=1) as persist,
            tc.tile_pool(name="stage", bufs=4) as stage,
            tc.tile_pool(name="small", bufs=1) as small,
            tc.tile_pool(name="psum", bufs=4, space="PSUM") as psum_pool,
        ):
            PT = 1024  # psum tile width (2 banks)
            CPT = PT // CHUNK  # chunks per psum tile
            TPH = HALF // CPT  # psum tiles per half-pass
            # Persistent operands
            tt_sb = [
                persist.tile([P, M_PTS], f32r, tag=f"tt{h}", name=f"tt{h}")
                for h in range(2)
            ]
            st_sb = [
                persist.tile([P, N_SHARD], f32r, tag=f"st{h}", name=f"st{h}")
                for h in range(2)
            ]
            nsqt_sb = persist.tile([1, M_PTS], f32r, tag="nsqt")
            ones_sb = persist.tile([1, P], f32r, tag="ones")
            onecol_sb = persist.tile([P, 1], f32, tag="onecol")
            zero_bias = persist.tile([P, 1], f32, tag="zero_bias")
            hinge_bias = persist.tile([P, 1], f32, tag="hinge_bias")
            sqs_sb = persist.tile([P, N_BLOCKS], f32, tag="sqs")
            # top-8 zneg candidates for every block
            cand = persist.tile([P, N_BLOCKS * 8], f32, tag="cand")

            # Small operands first so the first matmuls and the rank-1 fold
            # aren't queued behind 8MB of target data.
            nc.sync.dma_start(nsqt_sb[:, :], nsqt[:, :].bitcast(f32r))
            nc.sync.dma_start(ones_sb[:, :], ones1[:, :].bitcast(f32r))
            for h in range(2):
                nc.sync.dma_start(st_sb[h][:, :], st[h * P : (h + 1) * P, :].bitcast(f32r))
            nc.sync.dma_start(sqs_sb[:, :], sqs[:, :])
            nc.vector.memset(onecol_sb[:, :], 1.0)
            nc.vector.memset(zero_bias[:, :], 0.0)
            nc.vector.memset(hinge_bias[:, :], -HINGE)
            # Warm the PE (HAM clock gate) while target data streams in:
            # dummy K=1 matmuls on the small already-loaded tiles.
            warm = psum_pool.tile([P, PT], f32, tag="pz", name="warm")
            for w in range(8):
                nc.tensor.matmul(
                    warm[:, 0:CHUNK],
                    ones_sb[0:1, :],
                    nsqt_sb[0:1, 0:CHUNK],
                    start=True,
                    stop=True,
                )
            # Pre-load the ACT function tables (Sqrt/Relu) while ACT is idle.
            act_scratch = small.tile([P, 1], f32, tag="act_scratch")
            nc.scalar.activation(
                act_scratch[:, :], zero_bias[:, :], AF.Sqrt, bias=zero_bias[:, :]
            )
            nc.scalar.activation(
                act_scratch[:, :], zero_bias[:, :], AF.Relu, bias=zero_bias[:, :]
            )
            # Target halves in phase order: the whole half-0 phase (all 8
            # blocks) only needs tt[:, 0:4096], so DMA has the entire first
            # phase (~50us) to deliver the second half. No DMA-paced PE stalls
            # after the first ~1.3MB.
            # -|t|^2 replicated across all partitions for the free-dim
            # subtract (DMA broadcast: 0-stride partition read from DRAM).
            # Split in two and interleaved with the tt loads so the first
            # DVE-subtract group doesn't wait on a single 4MB transfer.
            nsqt_rep = persist.tile([P, M_PTS], f32, tag="nsqt_rep")
            for half in range(2):
                for j in range(half * 4096, half * 4096 + 4096, 2048):
                    for h in range(2):
                        nc.sync.dma_start(
                            tt_sb[h][:, j : j + 2048],
                            tt[h * P : (h + 1) * P, j : j + 2048].bitcast(f32r),
                        )
                rs = slice(half * 4096, (half + 1) * 4096)
                nc.sync.dma_start(
                    nsqt_rep[:, rs],
                    nsqt[:, rs].to_broadcast([P, 4096]),
                )

            cand16 = [
                persist.tile([P, 16], f32, tag=f"cand16_{b}", name=f"cand16_{b}")
                for b in range(N_BLOCKS)
            ]
            negd2 = small.tile([P, N_BLOCKS * 8], f32, tag="negd2")
            for half in range(N_CHUNKS // HALF):
                for b in range(N_BLOCKS):
                    bs = slice(b * P, (b + 1) * P)
                    # 5/8 of groups fold -|t|^2 on the PE (rank-1 matmul
                    # pass); the rest subtract on the DVE. Balances the two
                    # engines under the PE clock governor.
                    gi = half * N_BLOCKS + b
                    # Non-fold (DVE-subtract) groups need ~8.8us of DVE work
                    # vs ~6.9us of PE work, so spread them out -- consecutive
                    # heavy groups stall the zrow/PSUM pipeline. Half 1 gets
                    # one fewer so the DVE drains before the final block.
                    pe_fold = b not in (1, 3, 5) if half == 0 else b not in (1, 3)
                    zrow = stage.tile([P, HALF * CHUNK], f32, tag="zrow")
                    pts = [
                        psum_pool.tile([P, PT], f32, tag="pz", name=f"pz_{b}_{half}_{i}")
                        for i in range(TPH)
                    ]
                    # per-tile pass ordering: each PSUM tile finishes all its
                    # passes before the next, so ACT copies start ~4x earlier
                    for ti in range(TPH):
                        for src_h in range(2):
                            for j in range(CPT):
                                i = ti * CPT + j
                                c = half * HALF + i
                                cs = slice(c * CHUNK, (c + 1) * CHUNK)
                                nc.tensor.matmul(
                                    pts[ti][:, j * CHUNK : (j + 1) * CHUNK],
                                    st_sb[src_h][:, bs],
                                    tt_sb[src_h][:, cs],
                                    start=(src_h == 0),
                                    stop=(src_h == 1 and not pe_fold),
                                )
                        if pe_fold:
                            for j in range(CPT):
                                i = ti * CPT + j
                                c = half * HALF + i
                                cs = slice(c * CHUNK, (c + 1) * CHUNK)
                                nc.tensor.matmul(
                                    pts[ti][:, j * CHUNK : (j + 1) * CHUNK],
                                    ones_sb[0:1, :],
                                    nsqt_sb[0:1, cs],
                                    start=False,
                                    stop=True,
                                )
                    for i in range(TPH):
                        nc.scalar.copy(
                            out=zrow[:, i * PT : (i + 1) * PT], in_=pts[i][:, :]
                        )
                    if not pe_fold:
                        nc.vector.tensor_tensor(
                            out=zrow[:, :],
                            in0=zrow[:, :],
                            in1=nsqt_rep[:, half * HALF * CHUNK :][:, : HALF * CHUNK],
                            op=ALU.add,
                        )
                    # top-8 of this half-row as soon as its values land
                    if half == 1 and b >= N_BLOCKS - 2:
                        # final block: per-tile max8 so the last DVE piece is
                        # ~1.2us after the last copy instead of a 4.4us
                        # full-row max8
                        c32 = stage.tile([P, 4 * 8], f32, tag="c32", name=f"c32_last_{b}")
                        for i in range(TPH):
                            nc.vector.max(
                                out=c32[:, i * 8 : (i + 1) * 8],
                                in_=zrow[:, i * PT : (i + 1) * PT],
                            )
                        nc.vector.max(
                            out=cand16[b][:, half * 8 : (half + 1) * 8],
                            in_=c32[:, 0 : 4 * 8],
                        )
                    else:
                        nc.vector.max(
                            out=cand16[b][:, half * 8 : (half + 1) * 8],
                            in_=zrow[:, :],
                        )
                    if half == 1:
                        # eager per-block finalize: merge halves and compute
                        # negd2 = min(zneg - |s|^2, 0) == -max(d^2, 0)
                        nc.vector.max(
                            out=cand[:, b * 8 : (b + 1) * 8],
                            in_=cand16[b][:, :],
                        )
                        nc.vector.tensor_scalar(
                            out=negd2[:, b * 8 : (b + 1) * 8],
                            in0=cand[:, b * 8 : (b + 1) * 8],
                            scalar1=sqs_sb[:, b : b + 1],
                            scalar2=0.0,
                            op0=ALU.subtract,
                            op1=ALU.min,
                        )
            # dist = sqrt(-negd2); hinge = relu(dist - HINGE)
            dist = small.tile([P, N_BLOCKS * 8], f32, tag="dist")
            nc.scalar.activation(
                dist[:, :], negd2[:, :], AF.Sqrt, bias=zero_bias[:, :], scale=-1.0
            )
            # hinge = max(d - H, 0) == max(d, H) - H, fused on the DVE to
            # avoid an extra ACT hop in the tail chain
            hinge = small.tile([P, N_BLOCKS * 8], f32, tag="hinge")
            nc.vector.tensor_scalar(
                out=hinge[:, :],
                in0=dist[:, :],
                scalar1=HINGE,
                scalar2=HINGE,
                op0=ALU.max,
                op1=ALU.subtract,
            )
            # per-partition sum of the first TOP_K of each block's 8 candidates
            hv = hinge[:, :].rearrange("p (b k) -> p b k", k=8)
            psums = small.tile([P, 1], f32, tag="psums")
            nc.vector.reduce_sum(
                psums[:, :], hv[:, :, 0:k], axis=mybir.AxisListType.XY
            )
            # partition reduce: [1,1] = psums.T @ ones
            pfin = psum_pool.tile([1, 1], f32, tag="pz")
            nc.tensor.matmul(
                pfin[:, :], psums[:, :], onecol_sb[:, :], start=True, stop=True
            )
            out_sb = small.tile([1, 1], f32, tag="outsb")
            nc.scalar.copy(out=out_sb[:, :], in_=pfin[:, :])
            nc.sync.dma_start(out[:, :], out_sb[:, :])

    nc.compile()
    _CACHE[k] = nc
    return nc


def _prep_inputs(source, target):
    src = np.asarray(source, dtype=np.float32)
    tgt = np.asarray(target, dtype=np.float32)
    st2 = np.ascontiguousarray(src.T * np.float32(2.0))  # [D, N]
    ttt = np.ascontiguousarray(tgt.T)  # [D, M]
    sqt = (tgt.astype(np.float64) ** 2).sum(axis=1).astype(np.float32)
    nsqt = np.ascontiguousarray(-sqt[None, :])  # [1, M]
    sqs = (src.astype(np.float64) ** 2).sum(axis=1).astype(np.float32)  # [N]

    in_maps = []
    for c in range(N_CORES):
        lo = c * N_SHARD
        hi = lo + N_SHARD
        sqs_c = np.ascontiguousarray(
            sqs[lo:hi].reshape(N_BLOCKS, P).T
        )  # [P, N_BLOCKS]
        in_maps.append(
            {
                "st": np.ascontiguousarray(st2[:, lo:hi]),
                "tt": ttt,
                "nsqt": nsqt,
                "ones1": np.ones((1, P), dtype=np.float32),
                "sqs": sqs_c,
            }
        )
    return in_maps


def run_spmd(in_maps, trace=False, k=TOP_K, **kwargs):
    from concourse.bass_utils import run_bass_kernel_spmd

    nc = _build_nc(k)
    return run_bass_kernel_spmd(
        nc, in_maps, list(range(N_CORES)), trace=trace, **kwargs
    )


def kernel(source, target, top_k):
    k = int(top_k)
    assert 1 <= k <= 8, f"kernel supports top_k in [1, 8], got {k}"
    in_maps = _prep_inputs(source, target)
    res = run_spmd(in_maps, k=k)
    total = float(sum(float(r["out"][0, 0]) for r in res.results))
    return np.float32(total / (N_PTS * k))


# revision 63
# speedup vs baseline: 1.1939x; 1.0041x over previous
"""Trainium2 Bass kernel for nn_Density_loss (retrieval_knn).

Computes: mean over all (row, k) of max(topk_smallest_dist(source, target)[row, k] - 0.01, 0)
where dist is the Euclidean cdist via the Gram trick (matching the reference).

Strategy (8 NeuronCores, SPMD):
  - Shard source rows across the 8 cores (1024 rows each); every core holds the
    full target set.
  - Host pre-transposes operands (fp32 has no DMA-transpose path on TRN2) and
    pre-scales source by 2 so the PE directly produces 2*s.t.
  - Per core: Gram matmul in float32r (full fp32 precision at ~1 cycle/row for
    moving dim >= 256). The -|t|^2 term is folded in two ways, balancing the
    PE and DVE under the PE clock governor: ~5/8 of groups add it on the PE
    via a K=1 rank-1 matmul into the same PSUM accumulation group; the rest
    subtract a replicated |t|^2 tile on the DVE after the PSUM->SBUF copy.
  - Top-k: nc.vector.max (top-8 per partition, descending) per half-row
    [128, 4096], then a merge max8 over the 16 half-candidates per block
    => 8 smallest d^2 per source row; keep the first top_k=5.
  - Finalize on 128x64 candidates: d2 = |s|^2 - zneg (clamped at 0), sqrt,
    hinge relu, masked sum, partition-reduce via a ones-matmul.
  - Host sums the 8 per-core partials and divides by N*top_k.
"""

import sys

import numpy as np

for _p in (
    "/root/.axon_site",
    "/root/.axon_site/_ro/trn_rl_repo",
    "/root/.axon_site/_ro/pypackages",
    "/opt/trn_rl_repo",
):
    if _p not in sys.path:
        sys.path.append(_p)

N_PTS = 8192  # source rows
M_PTS = 8192  # target rows
D = 256  # feature dim
N_CORES = 8
N_SHARD = N_PTS // N_CORES  # 1024 source rows per core
P = 128  # SBUF partitions
N_BLOCKS = N_SHARD // P  # 8 row blocks per core
CHUNK = 512  # matmul moving-operand chunk (PSUM bank)
N_CHUNKS = M_PTS // CHUNK  # 16
HALF = 8  # chunks per PSUM half-pass (8 banks)
TOP_K = 5
HINGE = 0.01

_CACHE = {}


def _build_nc(k=TOP_K):
    if k in _CACHE:
        return _CACHE[k]

    import concourse.mybir as mybir
    from concourse import bacc
    from concourse.tile import TileContext

    f32 = mybir.dt.float32
    f32r = mybir.dt.float32r
    AF = mybir.ActivationFunctionType
    ALU = mybir.AluOpType

    nc = bacc.Bacc(trn_type="TRN2", target_bir_lowering=False, debug=False)

    st = nc.dram_tensor("st", [D, N_SHARD], f32, kind="ExternalInput")
    tt = nc.dram_tensor("tt", [D, M_PTS], f32, kind="ExternalInput")
    nsqt = nc.dram_tensor("nsqt", [1, M_PTS], f32, kind="ExternalInput")
    ones1 = nc.dram_tensor("ones1", [1, P], f32, kind="ExternalInput")
    sqs = nc.dram_tensor("sqs", [P, N_BLOCKS], f32, kind="ExternalInput")
    out = nc.dram_tensor("out", [1, 1], f32, kind="ExternalOutput")

    with TileContext(nc) as tc:
        with (
            tc.tile_pool(name="persist", bufs=1) as persist,
            tc.tile_pool(name="stage", bufs=4) as stage,
            tc.tile_pool(name="small", bufs=1) as small,
            tc.tile_pool(name="psum", bufs=4, space="PSUM") as psum_pool,
        ):
            PT = 1024  # psum tile width (2 banks)
            CPT = PT // CHUNK  # chunks per psum tile
            TPH = HALF // CPT  # psum tiles per half-pass
            # Persistent operands
            tt_sb = [
                persist.tile([P, M_PTS], f32r, tag=f"tt{h}", name=f"tt{h}")
                for h in range(2)
            ]
            st_sb = [
                persist.tile([P, N_SHARD], f32r, tag=f"st{h}", name=f"st{h}")
                for h in range(2)
            ]
            nsqt_sb = persist.tile([1, M_PTS], f32r, tag="nsqt")
            ones_sb = persist.tile([1, P], f32r, tag="ones")
            onecol_sb = persist.tile([P, 1], f32, tag="onecol")
            zero_bias = persist.tile([P, 1], f32, tag="zero_bias")
            hinge_bias = persist.tile([P, 1], f32, tag="hinge_bias")
            sqs_sb = persist.tile([P, N_BLOCKS], f32, tag="sqs")
            # top-8 zneg candidates for every block
            cand = persist.tile([P, N_BLOCKS * 8], f32, tag="cand")

            # Small operands first so the first matmuls and the rank-1 fold
            # aren't queued behind 8MB of target data.
            nc.sync.dma_start(nsqt_sb[:, :], nsqt[:, :].bitcast(f32r))
            nc.sync.dma_start(ones_sb[:, :], ones1[:, :].bitcast(f32r))
            for h in range(2):
                nc.sync.dma_start(st_sb[h][:, :], st[h * P : (h + 1) * P, :].bitcast(f32r))
            nc.sync.dma_start(sqs_sb[:, :], sqs[:, :])
            nc.vector.memset(onecol_sb[:, :], 1.0)
            nc.vector.memset(zero_bias[:, :], 0.0)
            nc.vector.memset(hinge_bias[:, :], -HINGE)
            # Warm the PE (HAM clock gate) while target data streams in:
            # dummy K=1 matmuls on the small already-loaded tiles.
            warm = psum_pool.tile([P, PT], f32, tag="pz", name="warm")
            for w in range(8):
                nc.tensor.matmul(
                    warm[:, 0:CHUNK],
                    ones_sb[0:1, :],
                    nsqt_sb[0:1, 0:CHUNK],
                    start=True,
                    stop=True,
                )
            # Pre-load the ACT function tables (Sqrt/Relu) while ACT is idle.
            act_scratch = small.tile([P, 1], f32, tag="act_scratch")
            nc.scalar.activation(
                act_scratch[:, :], zero_bias[:, :], AF.Sqrt, bias=zero_bias[:, :]
            )
            nc.scalar.activation(
                act_scratch[:, :], zero_bias[:, :], AF.Relu, bias=zero_bias[:, :]
            )
            # Target halves in phase order: the whole half-0 phase (all 8
            # blocks) only needs tt[:, 0:4096], so DMA has the entire first
            # phase (~50us) to deliver the second half. No DMA-paced PE stalls
            # after the first ~1.3MB.
            # -|t|^2 replicated across all partitions for the free-dim
            # subtract (DMA broadcast: 0-stride partition read from DRAM).
            # Split in two and interleaved with the tt loads so the first
            # DVE-subtract group doesn't wait on a single 4MB transfer.
            nsqt_rep = persist.tile([P, M_PTS], f32, tag="nsqt_rep")
            for half in range(2):
                for j in range(half * 4096, half * 4096 + 4096, 2048):
                    for h in range(2):
                        nc.sync.dma_start(
                            tt_sb[h][:, j : j + 2048],
                            tt[h * P : (h + 1) * P, j : j + 2048].bitcast(f32r),
                        )
                rs = slice(half * 4096, (half + 1) * 4096)
                nc.sync.dma_start(
                    nsqt_rep[:, rs],
                    nsqt[:, rs].to_broadcast([P, 4096]),
                )

            cand16 = [
                persist.tile([P, 16], f32, tag=f"cand16_{b}", name=f"cand16_{b}")
                for b in range(N_BLOCKS)
            ]
            negd2 = small.tile([P, N_BLOCKS * 8], f32, tag="negd2")
            for half in range(N_CHUNKS // HALF):
                for b in range(N_BLOCKS):
                    bs = slice(b * P, (b + 1) * P)
                    # 5/8 of groups fold -|t|^2 on the PE (rank-1 matmul
                    # pass); the rest subtract on the DVE. Balances the two
                    # engines under the PE clock governor.
                    gi = half * N_BLOCKS + b
                    # Non-fold (DVE-subtract) groups need ~8.8us of DVE work
                    # vs ~6.9us of PE work, so spread them out -- consecutive
                    # heavy groups stall the zrow/PSUM pipeline. Half 1 gets
                    # one fewer so the DVE drains before the final block.
                    pe_fold = b not in (1, 3, 5) if half == 0 else b not in (1, 3)
                    zrow = stage.tile([P, HALF * CHUNK], f32, tag="zrow")
                    pts = [
                        psum_pool.tile([P, PT], f32, tag="pz", name=f"pz_{b}_{half}_{i}")
                        for i in range(TPH)
                    ]
                    # per-tile pass ordering: each PSUM tile finishes all its
                    # passes before the next, so ACT copies start ~4x earlier
                    for ti in range(TPH):
                        for src_h in range(2):
                            for j in range(CPT):
                                i = ti * CPT + j
                                c = half * HALF + i
                                cs = slice(c * CHUNK, (c + 1) * CHUNK)
                                nc.tensor.matmul(
                                    pts[ti][:, j * CHUNK : (j + 1) * CHUNK],
                                    st_sb[src_h][:, bs],
                                    tt_sb[src_h][:, cs],
                                    start=(src_h == 0),
                                    stop=(src_h == 1 and not pe_fold),
                                )
                        if pe_fold:
                            for j in range(CPT):
                                i = ti * CPT + j
                                c = half * HALF + i
                                cs = slice(c * CHUNK, (c + 1) * CHUNK)
                                nc.tensor.matmul(
                                    pts[ti][:, j * CHUNK : (j + 1) * CHUNK],
                                    ones_sb[0:1, :],
                                    nsqt_sb[0:1, cs],
                                    start=False,
                                    stop=True,
                                )
                    for i in range(TPH):
                        nc.scalar.copy(
                            out=zrow[:, i * PT : (i + 1) * PT], in_=pts[i][:, :]
                        )
                    if not pe_fold:
                        nc.vector.tensor_tensor(
                            out=zrow[:, :],
                            in0=zrow[:, :],
                            in1=nsqt_rep[:, half * HALF * CHUNK :][:, : HALF * CHUNK],
                            op=ALU.add,
                        )
                    # top-8 of this half-row as soon as its values land
                    if half == 1 and b >= N_BLOCKS - 2:
                        # final block: per-tile max8 so the last DVE piece is
                        # ~1.2us after the last copy instead of a 4.4us
                        # full-row max8
                        c32 = stage.tile([P, 4 * 8], f32, tag="c32", name=f"c32_last_{b}")
                        for i in range(TPH):
                            nc.vector.max(
                                out=c32[:, i * 8 : (i + 1) * 8],
                                in_=zrow[:, i * PT : (i + 1) * PT],
                            )
                        nc.vector.max(
                            out=cand16[b][:, half * 8 : (half + 1) * 8],
                            in_=c32[:, 0 : 4 * 8],
                        )
                    else:
                        nc.vector.max(
                            out=cand16[b][:, half * 8 : (half + 1) * 8],
                            in_=zrow[:, :],
                        )
                    if half == 1:
                        # eager per-block finalize: merge halves and compute
                        # negd2 = min(zneg - |s|^2, 0) == -max(d^2, 0)
                        nc.vector.max(
                            out=cand[:, b * 8 : (b + 1) * 8],
                            in_=cand16[b][:, :],
                        )
                        nc.vector.tensor_scalar(
                            out=negd2[:, b * 8 : (b + 1) * 8],
                            in0=cand[:, b * 8 : (b + 1) * 8],
                            scalar1=sqs_sb[:, b : b + 1],
                            scalar2=0.0,
                            op0=ALU.subtract,
                            op1=ALU.min,
                        )
            # dist = sqrt(-negd2); hinge = relu(dist - HINGE)
            dist = small.tile([P, N_BLOCKS * 8], f32, tag="dist")
            nc.scalar.activation(
                dist[:, :], negd2[:, :], AF.Sqrt, bias=zero_bias[:, :], scale=-1.0
            )
            # hinge = max(d - H, 0) == max(d, H) - H, fused on the DVE to
            # avoid an extra ACT hop in the tail chain
            hinge = small.tile([P, N_BLOCKS * 8], f32, tag="hinge")
            nc.vector.tensor_scalar(
                out=hinge[:, :],
                in0=dist[:, :],
                scalar1=HINGE,
                scalar2=HINGE,
                op0=ALU.max,
                op1=ALU.subtract,
            )
            # per-partition sum of the first TOP_K of each block's 8 candidates
            hv = hinge[:, :].rearrange("p (b k) -> p b k", k=8)
            psums = small.tile([P, 1], f32, tag="psums")
            nc.vector.reduce_sum(
                psums[:, :], hv[:, :, 0:k], axis=mybir.AxisListType.XY
            )
            # partition reduce: [1,1] = psums.T @ ones
            pfin = psum_pool.tile([1, 1], f32, tag="pz")
            nc.tensor.matmul(
                pfin[:, :], psums[:, :], onecol_sb[:, :], start=True, stop=True
            )
            out_sb = small.tile([1, 1], f32, tag="outsb")
            nc.scalar.copy(out=out_sb[:, :], in_=pfin[:, :])
            nc.sync.dma_start(out[:, :], out_sb[:, :])

    nc.compile()
    _CACHE[k] = nc
    return nc


def _prep_inputs(source, target):
    src = np.asarray(source, dtype=np.float32)
    tgt = np.asarray(target, dtype=np.float32)
    st2 = np.ascontiguousarray(src.T * np.float32(2.0))  # [D, N]
    ttt = np.ascontiguousarray(tgt.T)  # [D, M]
    sqt = (tgt.astype(np.float64) ** 2).sum(axis=1).astype(np.float32)
    nsqt = np.ascontiguousarray(-sqt[None, :])  # [1, M]
    sqs = (src.astype(np.float64) ** 2).sum(axis=1).astype(np.float32)  # [N]

    in_maps = []
    for c in range(N_CORES):
        lo = c * N_SHARD
        hi = lo + N_SHARD
        sqs_c = np.ascontiguousarray(
            sqs[lo:hi].reshape(N_BLOCKS, P).T
        )  # [P, N_BLOCKS]
        in_maps.append(
            {
                "st": np.ascontiguousarray(st2[:, lo:hi]),
                "tt": ttt,
                "nsqt": nsqt,
                "ones1": np.ones((1, P), dtype=np.float32),
                "sqs": sqs_c,
            }
        )
    return in_maps


def run_spmd(in_maps, trace=False, k=TOP_K, **kwargs):
    from concourse.bass_utils import run_bass_kernel_spmd

    nc = _build_nc(k)
    return run_bass_kernel_spmd(
        nc, in_maps, list(range(N_CORES)), trace=trace, **kwargs
    )


def kernel(source, target, top_k):
    k = int(top_k)
    assert 1 <= k <= 8, f"kernel supports top_k in [1, 8], got {k}"
    in_maps = _prep_inputs(source, target)
    res = run_spmd(in_maps, k=k)
    total = float(sum(float(r["out"][0, 0]) for r in res.results))
    return np.float32(total / (N_PTS * k))


# revision 64
# speedup vs baseline: 1.2000x; 1.0051x over previous
"""Trainium2 Bass kernel for nn_Density_loss (retrieval_knn).

Computes: mean over all (row, k) of max(topk_smallest_dist(source, target)[row, k] - 0.01, 0)
where dist is the Euclidean cdist via the Gram trick (matching the reference).

Strategy (8 NeuronCores, SPMD):
  - Shard source rows across the 8 cores (1024 rows each); every core holds the
    full target set.
  - Host pre-transposes operands (fp32 has no DMA-transpose path on TRN2) and
    pre-scales source by 2 so the PE directly produces 2*s.t.
  - Per core: Gram matmul in float32r (full fp32 precision at ~1 cycle/row for
    moving dim >= 256). The -|t|^2 term is folded in two ways, balancing the
    PE and DVE under the PE clock governor: ~5/8 of groups add it on the PE
    via a K=1 rank-1 matmul into the same PSUM accumulation group; the rest
    subtract a replicated |t|^2 tile on the DVE after the PSUM->SBUF copy.
  - Top-k: nc.vector.max (top-8 per partition, descending) per half-row
    [128, 4096], then a merge max8 over the 16 half-candidates per block
    => 8 smallest d^2 per source row; keep the first top_k=5.
  - Finalize on 128x64 candidates: d2 = |s|^2 - zneg (clamped at 0), sqrt,
    hinge relu, masked sum, partition-reduce via a ones-matmul.
  - Host sums the 8 per-core partials and divides by N*top_k.
"""

import sys

import numpy as np

for _p in (
    "/root/.axon_site",
    "/root/.axon_site/_ro/trn_rl_repo",
    "/root/.axon_site/_ro/pypackages",
    "/opt/trn_rl_repo",
):
    if _p not in sys.path:
        sys.path.append(_p)

N_PTS = 8192  # source rows
M_PTS = 8192  # target rows
D = 256  # feature dim
N_CORES = 8
N_SHARD = N_PTS // N_CORES  # 1024 source rows per core
P = 128  # SBUF partitions
N_BLOCKS = N_SHARD // P  # 8 row blocks per core
CHUNK = 512  # matmul moving-operand chunk (PSUM bank)
N_CHUNKS = M_PTS // CHUNK  # 16
HALF = 8  # chunks per PSUM half-pass (8 banks)
TOP_K = 5
HINGE = 0.01

_CACHE = {}


def _build_nc(k=TOP_K):
    if k in _CACHE:
        return _CACHE[k]

    import concourse.mybir as mybir
    from concourse import bacc
    from concourse.tile import TileContext

    f32 = mybir.dt.float32
    f32r = mybir.dt.float32r
    AF = mybir.ActivationFunctionType
    ALU = mybir.AluOpType

    nc = bacc.Bacc(trn_type="TRN2", target_bir_lowering=False, debug=False)

    st = nc.dram_tensor("st", [D, N_SHARD], f32, kind="ExternalInput")
    tt = nc.dram_tensor("tt", [D, M_PTS], f32, kind="ExternalInput")
    nsqt = nc.dram_tensor("nsqt", [1, M_PTS], f32, kind="ExternalInput")
    ones1 = nc.dram_tensor("ones1", [1, P], f32, kind="ExternalInput")
    sqs = nc.dram_tensor("sqs", [P, N_BLOCKS], f32, kind="ExternalInput")
    out = nc.dram_tensor("out", [1, 1], f32, kind="ExternalOutput")

    with TileContext(nc) as tc:
        with (
            tc.tile_pool(name="persist", bufs=1) as persist,
            tc.tile_pool(name="stage", bufs=4) as stage,
            tc.tile_pool(name="small", bufs=1) as small,
            tc.tile_pool(name="psum", bufs=4, space="PSUM") as psum_pool,
        ):
            PT = 1024  # psum tile width (2 banks)
            CPT = PT // CHUNK  # chunks per psum tile
            TPH = HALF // CPT  # psum tiles per half-pass
            # Persistent operands
            tt_sb = [
                persist.tile([P, M_PTS], f32r, tag=f"tt{h}", name=f"tt{h}")
                for h in range(2)
            ]
            st_sb = [
                persist.tile([P, N_SHARD], f32r, tag=f"st{h}", name=f"st{h}")
                for h in range(2)
            ]
            nsqt_sb = persist.tile([1, M_PTS], f32r, tag="nsqt")
            ones_sb = persist.tile([1, P], f32r, tag="ones")
            onecol_sb = persist.tile([P, 1], f32, tag="onecol")
            zero_bias = persist.tile([P, 1], f32, tag="zero_bias")
            hinge_bias = persist.tile([P, 1], f32, tag="hinge_bias")
            sqs_sb = persist.tile([P, N_BLOCKS], f32, tag="sqs")
            # top-8 zneg candidates for every block
            cand = persist.tile([P, N_BLOCKS * 8], f32, tag="cand")

            # Small operands first so the first matmuls and the rank-1 fold
            # aren't queued behind 8MB of target data.
            nc.sync.dma_start(nsqt_sb[:, :], nsqt[:, :].bitcast(f32r))
            nc.sync.dma_start(ones_sb[:, :], ones1[:, :].bitcast(f32r))
            for h in range(2):
                nc.sync.dma_start(st_sb[h][:, :], st[h * P : (h + 1) * P, :].bitcast(f32r))
            nc.sync.dma_start(sqs_sb[:, :], sqs[:, :])
            nc.vector.memset(onecol_sb[:, :], 1.0)
            nc.vector.memset(zero_bias[:, :], 0.0)
            nc.vector.memset(hinge_bias[:, :], -HINGE)
            # Warm the PE (HAM clock gate) while target data streams in:
            # dummy K=1 matmuls on the small already-loaded tiles.
            warm = psum_pool.tile([P, PT], f32, tag="pz", name="warm")
            for w in range(8):
                nc.tensor.matmul(
                    warm[:, 0:CHUNK],
                    ones_sb[0:1, :],
                    nsqt_sb[0:1, 0:CHUNK],
                    start=True,
                    stop=True,
                )
            # Pre-load the ACT function tables (Sqrt/Relu) while ACT is idle.
            act_scratch = small.tile([P, 1], f32, tag="act_scratch")
            nc.scalar.activation(
                act_scratch[:, :], zero_bias[:, :], AF.Sqrt, bias=zero_bias[:, :]
            )
            nc.scalar.activation(
                act_scratch[:, :], zero_bias[:, :], AF.Relu, bias=zero_bias[:, :]
            )
            # Target halves in phase order: the whole half-0 phase (all 8
            # blocks) only needs tt[:, 0:4096], so DMA has the entire first
            # phase (~50us) to deliver the second half. No DMA-paced PE stalls
            # after the first ~1.3MB.
            # -|t|^2 replicated across all partitions for the free-dim
            # subtract (DMA broadcast: 0-stride partition read from DRAM).
            # Split in two and interleaved with the tt loads so the first
            # DVE-subtract group doesn't wait on a single 4MB transfer.
            nsqt_rep = persist.tile([P, M_PTS], f32, tag="nsqt_rep")
            for half in range(2):
                for j in range(half * 4096, half * 4096 + 4096, 2048):
                    for h in range(2):
                        nc.sync.dma_start(
                            tt_sb[h][:, j : j + 2048],
                            tt[h * P : (h + 1) * P, j : j + 2048].bitcast(f32r),
                        )
                rs = slice(half * 4096, (half + 1) * 4096)
                nc.sync.dma_start(
                    nsqt_rep[:, rs],
                    nsqt[:, rs].to_broadcast([P, 4096]),
                )

            cand16 = [
                persist.tile([P, 16], f32, tag=f"cand16_{b}", name=f"cand16_{b}")
                for b in range(N_BLOCKS)
            ]
            negd2 = small.tile([P, N_BLOCKS * 8], f32, tag="negd2")
            for half in range(N_CHUNKS // HALF):
                for b in range(N_BLOCKS):
                    bs = slice(b * P, (b + 1) * P)
                    # 5/8 of groups fold -|t|^2 on the PE (rank-1 matmul
                    # pass); the rest subtract on the DVE. Balances the two
                    # engines under the PE clock governor.
                    gi = half * N_BLOCKS + b
                    # Non-fold (DVE-subtract) groups need ~8.8us of DVE work
                    # vs ~6.9us of PE work, so spread them out -- consecutive
                    # heavy groups stall the zrow/PSUM pipeline. Half 1 gets
                    # one fewer so the DVE drains before the final block.
                    pe_fold = b not in (1, 3, 5) if half == 0 else b not in (1, 3)
                    zrow = stage.tile([P, HALF * CHUNK + 8], f32, tag="zrow")
                    pts = [
                        psum_pool.tile([P, PT], f32, tag="pz", name=f"pz_{b}_{half}_{i}")
                        for i in range(TPH)
                    ]
                    # per-tile pass ordering: each PSUM tile finishes all its
                    # passes before the next, so ACT copies start ~4x earlier
                    for ti in range(TPH):
                        for src_h in range(2):
                            for j in range(CPT):
                                i = ti * CPT + j
                                c = half * HALF + i
                                cs = slice(c * CHUNK, (c + 1) * CHUNK)
                                nc.tensor.matmul(
                                    pts[ti][:, j * CHUNK : (j + 1) * CHUNK],
                                    st_sb[src_h][:, bs],
                                    tt_sb[src_h][:, cs],
                                    start=(src_h == 0),
                                    stop=(src_h == 1 and not pe_fold),
                                )
                        if pe_fold:
                            for j in range(CPT):
                                i = ti * CPT + j
                                c = half * HALF + i
                                cs = slice(c * CHUNK, (c + 1) * CHUNK)
                                nc.tensor.matmul(
                                    pts[ti][:, j * CHUNK : (j + 1) * CHUNK],
                                    ones_sb[0:1, :],
                                    nsqt_sb[0:1, cs],
                                    start=False,
                                    stop=True,
                                )
                    for i in range(TPH):
                        nc.scalar.copy(
                            out=zrow[:, i * PT : (i + 1) * PT], in_=pts[i][:, :]
                        )
                    if not pe_fold:
                        nc.vector.tensor_tensor(
                            out=zrow[:, : HALF * CHUNK],
                            in0=zrow[:, : HALF * CHUNK],
                            in1=nsqt_rep[:, half * HALF * CHUNK :][:, : HALF * CHUNK],
                            op=ALU.add,
                        )
                    # top-8 of this half-row as soon as its values land
                    if half == 1 and b >= N_BLOCKS - 2:
                        # final block: per-tile max8 so the last DVE piece is
                        # ~1.2us after the last copy instead of a 4.4us
                        # full-row max8
                        c32 = stage.tile([P, 4 * 8], f32, tag="c32", name=f"c32_last_{b}")
                        for i in range(TPH):
                            nc.vector.max(
                                out=c32[:, i * 8 : (i + 1) * 8],
                                in_=zrow[:, i * PT : (i + 1) * PT],
                            )
                        nc.vector.max(
                            out=cand16[b][:, half * 8 : (half + 1) * 8],
                            in_=c32[:, 0 : 4 * 8],
                        )
                    elif half == 0:
                        nc.vector.max(
                            out=cand16[b][:, 0:8],
                            in_=zrow[:, : HALF * CHUNK],
                        )
                    else:
                        # fold the half-merge into this max8: tail 8 columns
                        # carry the half-0 candidates
                        nc.vector.tensor_copy(
                            out=zrow[:, HALF * CHUNK : HALF * CHUNK + 8],
                            in_=cand16[b][:, 0:8],
                        )
                        nc.vector.max(
                            out=cand[:, b * 8 : (b + 1) * 8],
                            in_=zrow[:, :],
                        )
                        nc.vector.tensor_scalar(
                            out=negd2[:, b * 8 : (b + 1) * 8],
                            in0=cand[:, b * 8 : (b + 1) * 8],
                            scalar1=sqs_sb[:, b : b + 1],
                            scalar2=0.0,
                            op0=ALU.subtract,
                            op1=ALU.min,
                        )
                    if half == 1 and b >= N_BLOCKS - 2:
                        # eager per-block finalize: merge halves and compute
                        # negd2 = min(zneg - |s|^2, 0) == -max(d^2, 0)
                        nc.vector.max(
                            out=cand[:, b * 8 : (b + 1) * 8],
                            in_=cand16[b][:, :],
                        )
                        nc.vector.tensor_scalar(
                            out=negd2[:, b * 8 : (b + 1) * 8],
                            in0=cand[:, b * 8 : (b + 1) * 8],
                            scalar1=sqs_sb[:, b : b + 1],
                            scalar2=0.0,
                            op0=ALU.subtract,
                            op1=ALU.min,
                        )
            # dist = sqrt(-negd2); hinge = relu(dist - HINGE)
            dist = small.tile([P, N_BLOCKS * 8], f32, tag="dist")
            nc.scalar.activation(
                dist[:, :], negd2[:, :], AF.Sqrt, bias=zero_bias[:, :], scale=-1.0
            )
            # hinge = max(d - H, 0) == max(d, H) - H, fused on the DVE to
            # avoid an extra ACT hop in the tail chain
            hinge = small.tile([P, N_BLOCKS * 8], f32, tag="hinge")
            nc.vector.tensor_scalar(
                out=hinge[:, :],
                in0=dist[:, :],
                scalar1=HINGE,
                scalar2=HINGE,
                op0=ALU.max,
                op1=ALU.subtract,
            )
            # per-partition sum of the first TOP_K of each block's 8 candidates
            hv = hinge[:, :].rearrange("p (b k) -> p b k", k=8)
            psums = small.tile([P, 1], f32, tag="psums")
            nc.vector.reduce_sum(
                psums[:, :], hv[:, :, 0:k], axis=mybir.AxisListType.XY
            )
            # partition reduce: [1,1] = psums.T @ ones
            pfin = psum_pool.tile([1, 1], f32, tag="pz")
            nc.tensor.matmul(
                pfin[:, :], psums[:, :], onecol_sb[:, :], start=True, stop=True
            )
            out_sb = small.tile([1, 1], f32, tag="outsb")
            nc.scalar.copy(out=out_sb[:, :], in_=pfin[:, :])
            nc.sync.dma_start(out[:, :], out_sb[:, :])

    nc.compile()
    _CACHE[k] = nc
    return nc


def _prep_inputs(source, target):
    src = np.asarray(source, dtype=np.float32)
    tgt = np.asarray(target, dtype=np.float32)
    st2 = np.ascontiguousarray(src.T * np.float32(2.0))  # [D, N]
    ttt = np.ascontiguousarray(tgt.T)  # [D, M]
    sqt = (tgt.astype(np.float64) ** 2).sum(axis=1).astype(np.float32)
    nsqt = np.ascontiguousarray(-sqt[None, :])  # [1, M]
    sqs = (src.astype(np.float64) ** 2).sum(axis=1).astype(np.float32)  # [N]

    in_maps = []
    for c in range(N_CORES):
        lo = c * N_SHARD
        hi = lo + N_SHARD
        sqs_c = np.ascontiguousarray(
            sqs[lo:hi].reshape(N_BLOCKS, P).T
        )  # [P, N_BLOCKS]
        in_maps.append(
            {
                "st": np.ascontiguousarray(st2[:, lo:hi]),
                "tt": ttt,
                "nsqt": nsqt,
                "ones1": np.ones((1, P), dtype=np.float32),
                "sqs": sqs_c,
            }
        )
    return in_maps


def run_spmd(in_maps, trace=False, k=TOP_K, **kwargs):
    from concourse.bass_utils import run_bass_kernel_spmd

    nc = _build_nc(k)
    return run_bass_kernel_spmd(
        nc, in_maps, list(range(N_CORES)), trace=trace, **kwargs
    )


def kernel(source, target, top_k):
    k = int(top_k)
    assert 1 <= k <= 8, f"kernel supports top_k in [1, 8], got {k}"
    in_maps = _prep_inputs(source, target)
    res = run_spmd(in_maps, k=k)
    total = float(sum(float(r["out"][0, 0]) for r in res.results))
    return np.float32(total / (N_PTS * k))
